# revision 56
# baseline (speedup 1.0000x reference)
"""Trainium2 Bass kernel for CrossMultiHeadedSelfAttention.

Problem: B=2, SQ=SK=2048, D=1024, H=16, HD=64 cross-attention
  q = x @ Wq + bq ; k = enc @ Wk + bk ; v = enc @ Wv + bv   (per head)
  out = softmax(q k^T / sqrt(HD)) v  -> concat heads -> @ Wo + bo

Sharding: 8 cores = 2 batches x 4 head-groups (4 heads per core).
Each core computes a partial output projection over its 4 heads; the host
sums the 4 partials per batch and adds the constant term
(bo + sum_h bv_h @ Wo_h, exact because softmax rows sum to 1).

Device-side math (per core, bf16 matmuls, f32 accumulation):
  - x/enc pre-transposed + pre-cast to bf16 on the host; d-major tiles
  - qT/kT in [head-pair e (128) x seq] layout; v in [k, 4 heads, 65] with a
    ones column per head (softmax row-sums fall out of the attn@v matmul)
  - scoresT chunk = kT_h.T @ qT_h -> exp (scale=1/8, no max subtraction:
    scores ~ N(0,1) so exp is safe in f32/bf16)
  - attn@V FLIPPED vs the v1 kernel: lhsT = exp chunk [128k x 128q]
    (M=128 -> full PE output rate), rhs = v [128k x 65] (N=65/instr), PSUM
    accumulation over all 16 k-chunks into q-major ou tiles [128q x 65].
    This halves attn@V PE time vs the e-major orientation (N=512/instr).
  - normalization is now per-partition-scalar (rowsum is column 64 of ou):
    copy ou->SBUF, reciprocal, tensor_scalar multiplies -- no transposes
    through DRAM
  - PE transpose (identity matmul) turns the q-major normalized tile into
    the e-major stk layout the output projection needs (128 cycles/tile)
  - y = sum_pairs stk_pair.T @ Wo_pair (K=128), PSUM -> SBUF -> DMA

Schedule: ACT (exp, ~1us per k-chunk) paces the main loop; all projection
work is interleaved into PE slack via a deadline-driven task pump. attn@V
runs LAG k-chunks behind exp so early v-tiles have time to project.
"""

import sys

for _p in ("/opt/trn_rl_repo", "/root/.axon_site/_ro/trn_rl_repo"):
    if _p not in sys.path:
        sys.path.insert(0, _p)

from collections import deque

import numpy as np
import ml_dtypes

import concourse.bass as bass
import concourse.tile as tile
from concourse import bacc, mybir
from concourse.bass_utils import run_bass_kernel_spmd
from concourse.masks import make_identity

BF16 = mybir.dt.bfloat16
F32 = mybir.dt.float32
AF = mybir.ActivationFunctionType

B, S, D, H, HD = 2, 2048, 1024, 16, 64
NCORES = 8
HPC = 4          # heads per core
NPAIR = 2        # head pairs per core
DC = D // 128    # 8 d-chunks
KC = S // 128    # 16 k-chunks
NQB = 4          # q blocks of 512
QB = 512
NQT = QB // 128  # q tiles per block
LAG = 6          # attn@V trails exp by this many k-chunks
EXB = 8          # exp tile bufs (must exceed LAG+1)

_CACHE = {}


def _build_program():
    nc = bacc.Bacc("TRN2", target_bir_lowering=False, debug=False, num_devices=NCORES)

    xt = nc.dram_tensor("xt", [D, S], BF16, kind="ExternalInput").ap()
    et = nc.dram_tensor("et", [D, S], BF16, kind="ExternalInput").ap()
    wp = nc.dram_tensor("wp", [128, 4, 2048], BF16, kind="ExternalInput").ap()
    bq = nc.dram_tensor("bq", [128, NPAIR], F32, kind="ExternalInput").ap()
    bk = nc.dram_tensor("bk", [128, NPAIR], F32, kind="ExternalInput").ap()
    out = nc.dram_tensor("out", [S, D], F32, kind="ExternalOutput").ap()

    with tile.TileContext(nc) as tc:
        from contextlib import ExitStack

        with ExitStack() as ctx:
            wts = ctx.enter_context(tc.tile_pool(name="wts", bufs=1))
            big = ctx.enter_context(tc.tile_pool(name="big", bufs=1))

            # all four weight matrices ship as ONE packed dram tensor so the
            # scalar queue pays a single DMA init before x block 0
            wp_sb = wts.tile([128, 4, 2048], BF16, name="wp_sb")
            wk_sb = wp_sb[:, 0].rearrange("p (a b c) -> p a b c", a=NPAIR, b=DC)
            wq_sb = wp_sb[:, 1].rearrange("p (a b c) -> p a b c", a=NPAIR, b=DC)
            wv_sb = wp_sb[:, 2].rearrange("p (a b) -> p a b", a=DC)
            wo_sb = wp_sb[:, 3].rearrange("p (a b) -> p a b", a=NPAIR)
            bq_sb = wts.tile([128, NPAIR], F32, name="bq_sb")
            bk_sb = wts.tile([128, NPAIR], F32, name="bk_sb")
            ident = wts.tile([128, 128], BF16, name="ident")

            # load order tuned so the first k/q projections start ~14us:
            # sync queue carries only the enc column blocks; the scalar-queue
            # (idle until the first exp) carries weights + x
            xT = big.tile([128, DC, S], BF16, name="xT")
            eT = big.tile([128, DC, S], BF16, name="eT")

            def load_x_block(q, sb4):
                sl = slice(sb4 * QB, (sb4 + 1) * QB)
                srcx = bass.AP(tensor=xt.tensor, offset=sb4 * QB,
                               ap=[[S, 128], [128 * S, DC], [1, QB]])
                q.dma_start(xT[:, :, sl], srcx)

            # scalar queue gets ONLY the small pre-exp weights: a DMA slice
            # occupies the ACT engine until the transfer completes, so bulk
            # loads there would push exp(0) out by many us.
            # sync: x block 0 (gates q -> exp(0)) then the first enc block;
            # gpsimd SWDGE streams the remaining enc blocks in parallel.
            def load_e_block(q, sb4, c0=0, c1=QB):
                sl = slice(sb4 * QB + c0, sb4 * QB + c1)
                src = bass.AP(tensor=et.tensor, offset=sb4 * QB + c0,
                              ap=[[S, 128], [128 * S, DC], [1, c1 - c0]])
                q.dma_start(eT[:, :, sl], src)

            load_e_block(nc.sync, 0, 0, 128)
            load_e_block(nc.sync, 0, 128, QB)
            for sb4 in range(1, NQB):
                load_e_block(nc.sync, sb4)
            make_identity(nc, ident)
            nc.gpsimd.dma_start(bq_sb, bq)
            nc.gpsimd.dma_start(bk_sb, bk)
            nc.scalar.dma_start(wp_sb, wp)
            for sb4 in range(NQB):
                load_x_block(nc.scalar, sb4)

            # ---- PSUM pools: sc 2x2 banks + ou 2x1 + y 1x2 = 8 banks ----
            psc = ctx.enter_context(tc.tile_pool(name="psc", bufs=2, space="PSUM"))
            pou = ctx.enter_context(tc.tile_pool(name="pou", bufs=1, space="PSUM"))
            py = ctx.enter_context(tc.tile_pool(name="py", bufs=2, space="PSUM"))
            wk2 = ctx.enter_context(tc.tile_pool(name="wk2", bufs=2))
            expp = ctx.enter_context(tc.tile_pool(name="expp", bufs=EXB))

            qT = [big.tile([128, S], BF16, name=f"qT{p}") for p in range(NPAIR)]
            kT = [big.tile([128, S], BF16, name=f"kT{p}") for p in range(NPAIR)]
            v = [big.tile([128, HPC, 65], BF16, name=f"v{s}") for s in range(KC)]

            # PE p-state warm-up: ~3us of throwaway matmuls on the identity
            # ramp the array to full clock before the real projections land
            wu = psc.tile([128, 2, QB], F32, name="wu", tag="sc")
            for _ in range(26):
                nc.tensor.matmul(wu[:, 0, 0:128], ident, ident)

            # ---------- projection helpers (PE work fed into loop slack) ----
            # on_act=True drains the PSUM result via the ACT engine: during
            # the warm-up region ACT is starving anyway, and this keeps the
            # projection round-trip off the busy DVE queue.
            def project_k_chunk(p, sb4, c0=0, c1=QB, on_act=False):
                sl = slice(sb4 * QB + c0, sb4 * QB + c1)
                pk = py.tile([128, QB], F32, name="pk", tag="y")
                for d in range(DC):
                    nc.tensor.matmul(pk[:, 0:c1 - c0], wk_sb[:, p, d, :],
                                     eT[:, d, sl],
                                     start=(d == 0), stop=(d == DC - 1))
                if on_act:
                    nc.scalar.add(kT[p][:, sl], pk[:, 0:c1 - c0],
                                  bk_sb[:, p:p + 1])
                else:
                    nc.vector.tensor_scalar_add(kT[p][:, sl], pk[:, 0:c1 - c0],
                                                bk_sb[:, p:p + 1])

            def project_q(p, qb, on_act=False):
                qsl = slice(qb * QB, (qb + 1) * QB)
                pq = py.tile([128, QB], F32, name="pq", tag="y")
                for d in range(DC):
                    nc.tensor.matmul(pq, wq_sb[:, p, d, :], xT[:, d, qsl],
                                     start=(d == 0), stop=(d == DC - 1))
                if on_act:
                    nc.scalar.add(qT[p][:, qsl], pq, bq_sb[:, p:p + 1])
                else:
                    nc.vector.tensor_scalar_add(qT[p][:, qsl], pq,
                                                bq_sb[:, p:p + 1])

            def project_v_tile(s, on_act=False):
                pv = py.tile([128, 256], F32, name="pv", tag="y")
                for d in range(DC):
                    nc.tensor.matmul(pv, eT[:, d, s * 128:(s + 1) * 128],
                                     wv_sb[:, d, :],
                                     start=(d == 0), stop=(d == DC - 1))
                src = pv.rearrange("p (h e) -> p h e", h=HPC)
                if on_act:
                    nc.scalar.copy(v[s][:, :, 0:64], src)
                else:
                    nc.vector.tensor_copy(v[s][:, :, 0:64], src)
                nc.vector.memset(v[s][:, :, 64:65], 1.0)

            # ---------- deadline-driven task pump --------------------------
            tasks = []

            def add_task(due, cost, fn):
                tasks.append((due, cost, fn))

            def pump(now, room):
                while taskq:
                    due, cost, fn = taskq[0]
                    if due <= now or room > 0:
                        taskq.popleft()
                        fn()
                        room -= cost
                    else:
                        break
                return room

            # ---------- attention emission helpers -------------------------
            def emit_sc_exp(st, kc):
                p, qb = st["p"], st["qb"]
                ksl = slice(kc * 128, (kc + 1) * 128)
                qsl = slice(qb * QB, (qb + 1) * QB)
                sc = psc.tile([128, 2, QB], F32, name="sc", tag="sc")
                for h2 in range(2):
                    hp = slice(h2 * 64, (h2 + 1) * 64)
                    nc.tensor.matmul(sc[:, h2, :], kT[p][hp, ksl], qT[p][hp, qsl])
                ex = expp.tile([128, 2, QB], BF16, name="ex", tag="ex")
                nc.scalar.activation(ex, sc, AF.Exp, scale=0.125)
                st["exs"][kc] = ex

            def emit_av(st, kc):
                # NOTE: start=True zeroes the whole 2KB PSUM bank (the "zero
                # region"), so only the first matmul into each ou bank may
                # set it, and only the last may set stop.
                p = st["p"]
                ex = st["exs"].pop(kc)
                for h2 in range(2):
                    for qt in range(NQT):
                        nc.tensor.matmul(
                            st["ou"][h2][:, qt, :],
                            ex[:, h2, qt * 128:(qt + 1) * 128],
                            v[kc][:, 2 * p + h2, :],
                            start=(kc == 0 and qt == 0),
                            stop=(kc == KC - 1 and qt == NQT - 1))

            cur_stk = {}

            def emit_osb(st):
                # free the ou PSUM slots fast: copy to SBUF, then reciprocal
                # of the rowsum column
                osb = wk2.tile([128, 2, NQT, 65], F32, name="osb", tag="osb",
                               bufs=2)
                for h2 in range(2):
                    nc.vector.tensor_copy(osb[:, h2], st["ou"][h2])
                rr = wk2.tile([128, 2 * NQT, 1], F32, name="rr", tag="rr", bufs=2)
                nc.vector.reciprocal_approx_fast(
                    rr, osb.rearrange("p a b c -> p (a b) c")[:, :, 64:65])
                st["osb"], st["rr"] = osb, rr
                st["stk"] = wk2.tile([128, QB], BF16, name=f"stk{st['p']}",
                                     tag=f"stk{st['p']}", bufs=2)
                cur_stk[st["p"]] = st["stk"]

            def emit_norm(st, qt):
                onm = wk2.tile([128, 128], BF16, name="onm", tag="onm", bufs=4)
                for h2 in range(2):
                    nc.vector.tensor_scalar_mul(
                        onm[:, h2 * 64:(h2 + 1) * 64],
                        st["osb"][:, h2, qt, 0:64],
                        st["rr"][:, h2 * NQT + qt, :])
                st["onm"][qt] = onm

            def emit_tr(st, qt):
                onm = st["onm"].pop(qt)
                trp = py.tile([128, 128], BF16, name="trp", tag="y")
                nc.tensor.transpose(trp, onm, ident)
                nc.vector.tensor_copy(st["stk"][:, qt * 128:(qt + 1) * 128], trp)

            cur_ysb = [None]

            def emit_outproj_piece(qb, i, fast=False):
                # reads cur_stk at emission time: by the first piece (slot
                # LAG+2) both pairs' stk tiles for `qb` are registered.
                # fast mode (epilogue): spread yp over the now-idle sc PSUM
                # slots and alternate copy engines so the tail pipelines.
                qt, dc2 = divmod(i, 2)
                if dc2 == 0:
                    cur_ysb[0] = wk2.tile([128, D], F32, name="ysb", tag="ysb",
                                          bufs=2)
                ysb = cur_ysb[0]
                dsl = slice(dc2 * 512, (dc2 + 1) * 512)
                if fast and i % 2 == 0:
                    yp = psc.tile([128, 2, QB], F32, name="ype", tag="sc")
                    yp = yp[:, 0, :]
                else:
                    yp = py.tile([128, 512], F32, name="yp", tag="y")
                for p in range(NPAIR):
                    nc.tensor.matmul(yp, cur_stk[p][:, qt * 128:(qt + 1) * 128],
                                     wo_sb[:, p, dsl],
                                     start=(p == 0), stop=(p == NPAIR - 1))
                if fast and i % 2 == 1:
                    # ACT is idle once the last exp is done and can read PSUM
                    nc.scalar.copy(ysb[:, dsl], yp)
                else:
                    nc.vector.tensor_copy(ysb[:, dsl], yp)
                if dc2 == 1:
                    r0 = qb * QB + qt * 128
                    nc.sync.dma_start(out[r0:r0 + 128, :], ysb)

            # ---------- global task schedule -------------------------------
            KCOST, QCOST, VCOST = 4400, 4400, 2300
            # loop 0 = (qb0, p0): k chunks due before their sc; v tiles due
            # before their (lagged) av; pair-1 k0/q due late in loop 0
            add_task((0, 2), KCOST, lambda: project_k_chunk(0, 1))
            for s_ in range(2, 10):
                add_task((0, s_ + 5), VCOST, lambda s=s_: project_v_tile(s))
            add_task((0, 6), KCOST, lambda: project_k_chunk(0, 2))
            add_task((0, 10), KCOST, lambda: project_k_chunk(0, 3))
            add_task((0, 13), KCOST, lambda: project_k_chunk(1, 0))
            add_task((0, 14), QCOST, lambda: project_q(1, 0))
            for s_ in range(10, 16):
                due = (0, 15) if s_ == 10 else (1, s_ - 11)
                add_task(due, VCOST, lambda s=s_: project_v_tile(s))
            add_task((1, 2), KCOST, lambda: project_k_chunk(1, 1))
            add_task((1, 6), KCOST, lambda: project_k_chunk(1, 2))
            add_task((1, 10), KCOST, lambda: project_k_chunk(1, 3))
            add_task((1, 14), QCOST, lambda: project_q(0, 1))
            add_task((2, 14), QCOST, lambda: project_q(1, 1))
            add_task((3, 14), QCOST, lambda: project_q(0, 2))
            add_task((4, 14), QCOST, lambda: project_q(1, 2))
            add_task((5, 14), QCOST, lambda: project_q(0, 3))
            add_task((6, 14), QCOST, lambda: project_q(1, 3))
            tasks.sort(key=lambda t: t[0])
            taskq = deque(tasks)

            def make_state(p, qb):
                return {"p": p, "qb": qb, "exs": {}, "onm": {},
                        "ou": [pou.tile([128, NQT, 65], F32, name=f"ou{h2}",
                                        tag=f"ou{h2}", bufs=1)
                               for h2 in range(2)]}

            # ---------- prologue: k/v first (enc lands first), then q the
            # moment x block 0 arrives ------------------------------------
            project_k_chunk(0, 0, 0, 128)
            project_k_chunk(0, 0, 128, QB)
            project_v_tile(0)
            project_v_tile(1)
            project_q(0, 0)

            # ---------- main loops ----------------------------------------
            loops = [(qb, p) for qb in range(NQB) for p in range(NPAIR)]
            prev = None
            for L, (qb, p) in enumerate(loops):
                pump((L, -1), 0)
                outproj_qb = qb - 1 if (p == 0 and qb > 0) else None
                st = make_state(p, qb)
                emit_sc_exp(st, 0)
                last = L == len(loops) - 1
                debt = 0
                for kc in range(KC):
                    room = 1376 - debt
                    if kc < KC - 1:
                        emit_sc_exp(st, kc + 1)
                    if prev is not None and kc < LAG:
                        emit_av(prev, KC - LAG + kc)
                        room -= 520
                        if kc == LAG - 1:
                            emit_osb(prev)
                    if kc >= LAG:
                        emit_av(st, kc - LAG)
                        room -= 520
                    else:
                        room += 500
                    if prev is not None and LAG <= kc < LAG + 4:
                        emit_norm(prev, kc - LAG)
                        room -= 100
                    if prev is not None and LAG + 1 <= kc < LAG + 5:
                        emit_tr(prev, kc - LAG - 1)
                        room -= 200
                        if kc == LAG + 4:
                            prev = None
                    if outproj_qb is not None and LAG + 2 <= kc < LAG + 10:
                        emit_outproj_piece(outproj_qb, kc - LAG - 2)
                        room -= 1100
                    room = pump((L, kc), room)
                    if last and kc >= 11:
                        # catch-up: drain the av tail inside the loop so the
                        # epilogue starts almost immediately after exp(15)
                        emit_av(st, kc - 1)
                    debt = max(0, -room)
                prev = st

            # ---------- epilogue ------------------------------------------
            st = prev
            for kc in sorted(st["exs"]):
                emit_av(st, kc)
            emit_osb(st)
            for qt in range(NQT):
                emit_norm(st, qt)
                emit_tr(st, qt)
                emit_outproj_piece(NQB - 1, 2 * qt, fast=True)
                emit_outproj_piece(NQB - 1, 2 * qt + 1, fast=True)
            pump((99, 99), 0)

    nc.compile()
    return nc


def _bf16(a):
    return np.ascontiguousarray(a.astype(ml_dtypes.bfloat16))


def _host_prep(inputs):
    x = np.asarray(inputs["x"], np.float32)
    enc = np.asarray(inputs["encoder_output"], np.float32)
    Wq = np.asarray(inputs["Wq"], np.float32)
    bq = np.asarray(inputs["bq"], np.float32)
    Wk = np.asarray(inputs["Wk"], np.float32)
    bk = np.asarray(inputs["bk"], np.float32)
    Wv = np.asarray(inputs["Wv"], np.float32)
    Wo = np.asarray(inputs["Wo"], np.float32)

    xt_b = [_bf16(x[b].T) for b in range(B)]
    et_b = [_bf16(enc[b].T) for b in range(B)]

    in_maps = []
    for c in range(NCORES):
        b = c // 4
        hb = HPC * (c % 4)

        wq_c = Wq[hb:hb + 4].reshape(2, 2, DC, 128, HD)  # [pair, hw, dc, dp, e]
        wq_c = wq_c.transpose(3, 0, 2, 1, 4).reshape(128, NPAIR, DC, 128)
        wk_c = Wk[hb:hb + 4].reshape(2, 2, DC, 128, HD)
        wk_c = wk_c.transpose(3, 0, 2, 1, 4).reshape(128, NPAIR, DC, 128)
        wv_c = Wv[hb:hb + 4].reshape(4, DC, 128, HD)
        wv_c = wv_c.transpose(2, 1, 0, 3).reshape(128, DC, 256)
        wo_c = Wo[hb * HD:(hb + 4) * HD].reshape(2, 2, HD, D)  # [pair, hw, e, d]
        wo_c = wo_c.transpose(1, 2, 0, 3).reshape(128, NPAIR, D)
        bq_c = bq[hb:hb + 4].reshape(2, 2, HD).transpose(1, 2, 0).reshape(128, NPAIR)
        bk_c = bk[hb:hb + 4].reshape(2, 2, HD).transpose(1, 2, 0).reshape(128, NPAIR)

        wp_c = np.stack([wk_c.reshape(128, -1), wq_c.reshape(128, -1),
                         wv_c.reshape(128, -1), wo_c.reshape(128, -1)], axis=1)
        in_maps.append({
            "xt": xt_b[b],
            "et": et_b[b],
            "wp": _bf16(wp_c),
            "bq": np.ascontiguousarray(bq_c),
            "bk": np.ascontiguousarray(bk_c),
        })
    return in_maps


def kernel(**inputs):
    if "nc" not in _CACHE:
        _CACHE["nc"] = _build_program()
    nc = _CACHE["nc"]

    in_maps = _host_prep(inputs)
    res = None
    for attempt in range(3):
        try:
            res = run_bass_kernel_spmd(nc, in_maps, core_ids=list(range(NCORES)))
            break
        except Exception:
            if attempt == 2:
                raise
            import time
            time.sleep(5)
    _CACHE["last_results"] = res

    bv = np.asarray(inputs["bv"], np.float32)
    Wo = np.asarray(inputs["Wo"], np.float32)
    bo = np.asarray(inputs["bo"], np.float32)
    const_d = bo + np.einsum("he,hed->d", bv,
                             Wo.reshape(H, HD, D)).astype(np.float32)

    out = np.empty((B, S, D), np.float32)
    for b in range(B):
        acc = res.results[4 * b]["out"].astype(np.float32).copy()
        for c in range(4 * b + 1, 4 * b + 4):
            acc += res.results[c]["out"]
        out[b] = acc + const_d
    return out


# revision 57
# speedup vs baseline: 1.0204x; 1.0204x over previous
"""Trainium2 Bass kernel for CrossMultiHeadedSelfAttention.

Problem: B=2, SQ=SK=2048, D=1024, H=16, HD=64 cross-attention
  q = x @ Wq + bq ; k = enc @ Wk + bk ; v = enc @ Wv + bv   (per head)
  out = softmax(q k^T / sqrt(HD)) v  -> concat heads -> @ Wo + bo

Sharding: 8 cores = 2 batches x 4 head-groups (4 heads per core).
Each core computes a partial output projection over its 4 heads; the host
sums the 4 partials per batch and adds the constant term
(bo + sum_h bv_h @ Wo_h, exact because softmax rows sum to 1).

Device-side math (per core, bf16 matmuls, f32 accumulation):
  - x/enc pre-transposed + pre-cast to bf16 on the host; d-major tiles
  - qT/kT in [head-pair e (128) x seq] layout; v in [k, 4 heads, 65] with a
    ones column per head (softmax row-sums fall out of the attn@v matmul)
  - scoresT chunk = kT_h.T @ qT_h -> exp (scale=1/8, no max subtraction:
    scores ~ N(0,1) so exp is safe in f32/bf16)
  - attn@V FLIPPED vs the v1 kernel: lhsT = exp chunk [128k x 128q]
    (M=128 -> full PE output rate), rhs = v [128k x 65] (N=65/instr), PSUM
    accumulation over all 16 k-chunks into q-major ou tiles [128q x 65].
    This halves attn@V PE time vs the e-major orientation (N=512/instr).
  - normalization is now per-partition-scalar (rowsum is column 64 of ou):
    copy ou->SBUF, reciprocal, tensor_scalar multiplies -- no transposes
    through DRAM
  - PE transpose (identity matmul) turns the q-major normalized tile into
    the e-major stk layout the output projection needs (128 cycles/tile)
  - y = sum_pairs stk_pair.T @ Wo_pair (K=128), PSUM -> SBUF -> DMA

Schedule: ACT (exp, ~1us per k-chunk) paces the main loop; all projection
work is interleaved into PE slack via a deadline-driven task pump. attn@V
runs LAG k-chunks behind exp so early v-tiles have time to project.
"""

import sys

for _p in ("/opt/trn_rl_repo", "/root/.axon_site/_ro/trn_rl_repo"):
    if _p not in sys.path:
        sys.path.insert(0, _p)

from collections import deque

import numpy as np
import ml_dtypes

import concourse.bass as bass
import concourse.tile as tile
from concourse import bacc, mybir
from concourse.bass_utils import run_bass_kernel_spmd
from concourse.masks import make_identity

BF16 = mybir.dt.bfloat16
F32 = mybir.dt.float32
AF = mybir.ActivationFunctionType

B, S, D, H, HD = 2, 2048, 1024, 16, 64
NCORES = 8
HPC = 4          # heads per core
NPAIR = 2        # head pairs per core
DC = D // 128    # 8 d-chunks
KC = S // 128    # 16 k-chunks
NQB = 4          # q blocks of 512
QB = 512
NQT = QB // 128  # q tiles per block
LAG = 6          # attn@V trails exp by this many k-chunks
EXB = 8          # exp tile bufs (must exceed LAG+1)

_CACHE = {}


def _build_program():
    nc = bacc.Bacc("TRN2", target_bir_lowering=False, debug=False, num_devices=NCORES)

    xt = nc.dram_tensor("xt", [D, S], BF16, kind="ExternalInput").ap()
    et = nc.dram_tensor("et", [D, S], BF16, kind="ExternalInput").ap()
    wq = nc.dram_tensor("wq", [128, NPAIR, DC, 128], BF16, kind="ExternalInput").ap()
    wk = nc.dram_tensor("wk", [128, NPAIR, DC, 128], BF16, kind="ExternalInput").ap()
    wv = nc.dram_tensor("wv", [128, DC, 256], BF16, kind="ExternalInput").ap()
    wo = nc.dram_tensor("wo", [128, NPAIR, D], BF16, kind="ExternalInput").ap()
    bq = nc.dram_tensor("bq", [128, NPAIR], F32, kind="ExternalInput").ap()
    bk = nc.dram_tensor("bk", [128, NPAIR], F32, kind="ExternalInput").ap()
    out = nc.dram_tensor("out", [S, D], F32, kind="ExternalOutput").ap()

    with tile.TileContext(nc) as tc:
        from contextlib import ExitStack

        with ExitStack() as ctx:
            wts = ctx.enter_context(tc.tile_pool(name="wts", bufs=1))
            big = ctx.enter_context(tc.tile_pool(name="big", bufs=1))

            wq_sb = wts.tile([128, NPAIR, DC, 128], BF16, name="wq_sb")
            wk_sb = wts.tile([128, NPAIR, DC, 128], BF16, name="wk_sb")
            wv_sb = wts.tile([128, DC, 256], BF16, name="wv_sb")
            wo_sb = wts.tile([128, NPAIR, D], BF16, name="wo_sb")
            bq_sb = wts.tile([128, NPAIR], F32, name="bq_sb")
            bk_sb = wts.tile([128, NPAIR], F32, name="bk_sb")
            ident = wts.tile([128, 128], BF16, name="ident")

            # load order tuned so the first k/q projections start ~14us:
            # sync queue carries only the enc column blocks; the scalar-queue
            # (idle until the first exp) carries weights + x
            xT = big.tile([128, DC, S], BF16, name="xT")
            eT = big.tile([128, DC, S], BF16, name="eT")

            def load_x_block(q, sb4):
                sl = slice(sb4 * QB, (sb4 + 1) * QB)
                srcx = bass.AP(tensor=xt.tensor, offset=sb4 * QB,
                               ap=[[S, 128], [128 * S, DC], [1, QB]])
                q.dma_start(xT[:, :, sl], srcx)

            # scalar queue gets ONLY the small pre-exp weights: a DMA slice
            # occupies the ACT engine until the transfer completes, so bulk
            # loads there would push exp(0) out by many us.
            # sync: x block 0 (gates q -> exp(0)) then the first enc block;
            # gpsimd SWDGE streams the remaining enc blocks in parallel.
            def load_e_block(q, sb4, c0=0, c1=QB):
                sl = slice(sb4 * QB + c0, sb4 * QB + c1)
                src = bass.AP(tensor=et.tensor, offset=sb4 * QB + c0,
                              ap=[[S, 128], [128 * S, DC], [1, c1 - c0]])
                q.dma_start(eT[:, :, sl], src)

            load_e_block(nc.sync, 0, 0, 128)
            load_e_block(nc.sync, 0, 128, QB)
            for sb4 in range(1, NQB):
                load_e_block(nc.sync, sb4)
            for sb, dr in ((bq_sb, bq), (bk_sb, bk), (wk_sb, wk),
                           (wv_sb, wv), (wq_sb, wq)):
                nc.scalar.dma_start(sb, dr)
            load_x_block(nc.scalar, 0)
            nc.scalar.dma_start(wo_sb, wo)
            for sb4 in range(1, NQB):
                load_x_block(nc.scalar, sb4)
            make_identity(nc, ident)

            # ---- PSUM pools: sc 2x2 banks + ou 2x1 + y 1x2 = 8 banks ----
            psc = ctx.enter_context(tc.tile_pool(name="psc", bufs=2, space="PSUM"))
            pou = ctx.enter_context(tc.tile_pool(name="pou", bufs=1, space="PSUM"))
            py = ctx.enter_context(tc.tile_pool(name="py", bufs=2, space="PSUM"))
            wk2 = ctx.enter_context(tc.tile_pool(name="wk2", bufs=2))
            expp = ctx.enter_context(tc.tile_pool(name="expp", bufs=EXB))

            qT = [big.tile([128, S], BF16, name=f"qT{p}") for p in range(NPAIR)]
            kT = [big.tile([128, S], BF16, name=f"kT{p}") for p in range(NPAIR)]
            v = [big.tile([128, HPC, 65], BF16, name=f"v{s}") for s in range(KC)]

            # PE p-state warm-up: ~3us of throwaway matmuls on the identity
            # ramp the array to full clock before the real projections land
            wu = psc.tile([128, 2, QB], F32, name="wu", tag="sc")
            for _ in range(26):
                nc.tensor.matmul(wu[:, 0, 0:128], ident, ident)

            # ---------- projection helpers (PE work fed into loop slack) ----
            # on_act=True drains the PSUM result via the ACT engine: during
            # the warm-up region ACT is starving anyway, and this keeps the
            # projection round-trip off the busy DVE queue.
            def project_k_chunk(p, sb4, c0=0, c1=QB, on_act=False):
                sl = slice(sb4 * QB + c0, sb4 * QB + c1)
                pk = py.tile([128, QB], F32, name="pk", tag="y")
                for d in range(DC):
                    nc.tensor.matmul(pk[:, 0:c1 - c0], wk_sb[:, p, d, :],
                                     eT[:, d, sl],
                                     start=(d == 0), stop=(d == DC - 1))
                if on_act:
                    nc.scalar.add(kT[p][:, sl], pk[:, 0:c1 - c0],
                                  bk_sb[:, p:p + 1])
                else:
                    nc.vector.tensor_scalar_add(kT[p][:, sl], pk[:, 0:c1 - c0],
                                                bk_sb[:, p:p + 1])

            def project_q(p, qb, on_act=False):
                qsl = slice(qb * QB, (qb + 1) * QB)
                pq = py.tile([128, QB], F32, name="pq", tag="y")
                for d in range(DC):
                    nc.tensor.matmul(pq, wq_sb[:, p, d, :], xT[:, d, qsl],
                                     start=(d == 0), stop=(d == DC - 1))
                if on_act:
                    nc.scalar.add(qT[p][:, qsl], pq, bq_sb[:, p:p + 1])
                else:
                    nc.vector.tensor_scalar_add(qT[p][:, qsl], pq,
                                                bq_sb[:, p:p + 1])

            def project_v_tile(s, on_act=False):
                pv = py.tile([128, 256], F32, name="pv", tag="y")
                for d in range(DC):
                    nc.tensor.matmul(pv, eT[:, d, s * 128:(s + 1) * 128],
                                     wv_sb[:, d, :],
                                     start=(d == 0), stop=(d == DC - 1))
                src = pv.rearrange("p (h e) -> p h e", h=HPC)
                if on_act:
                    nc.scalar.copy(v[s][:, :, 0:64], src)
                else:
                    nc.vector.tensor_copy(v[s][:, :, 0:64], src)
                nc.vector.memset(v[s][:, :, 64:65], 1.0)

            # ---------- deadline-driven task pump --------------------------
            tasks = []

            def add_task(due, cost, fn):
                tasks.append((due, cost, fn))

            def pump(now, room):
                while taskq:
                    due, cost, fn = taskq[0]
                    if due <= now or room > 0:
                        taskq.popleft()
                        fn()
                        room -= cost
                    else:
                        break
                return room

            # ---------- attention emission helpers -------------------------
            def emit_sc_exp(st, kc):
                p, qb = st["p"], st["qb"]
                ksl = slice(kc * 128, (kc + 1) * 128)
                qsl = slice(qb * QB, (qb + 1) * QB)
                sc = psc.tile([128, 2, QB], F32, name="sc", tag="sc")
                for h2 in range(2):
                    hp = slice(h2 * 64, (h2 + 1) * 64)
                    nc.tensor.matmul(sc[:, h2, :], kT[p][hp, ksl], qT[p][hp, qsl])
                ex = expp.tile([128, 2, QB], BF16, name="ex", tag="ex")
                nc.scalar.activation(ex, sc, AF.Exp, scale=0.125)
                st["exs"][kc] = ex

            def emit_av(st, kc):
                # NOTE: start=True zeroes the whole 2KB PSUM bank (the "zero
                # region"), so only the first matmul into each ou bank may
                # set it, and only the last may set stop.
                p = st["p"]
                ex = st["exs"].pop(kc)
                for h2 in range(2):
                    for qt in range(NQT):
                        nc.tensor.matmul(
                            st["ou"][h2][:, qt, :],
                            ex[:, h2, qt * 128:(qt + 1) * 128],
                            v[kc][:, 2 * p + h2, :],
                            start=(kc == 0 and qt == 0),
                            stop=(kc == KC - 1 and qt == NQT - 1))

            cur_stk = {}

            def emit_osb(st):
                # free the ou PSUM slots fast: copy to SBUF, then reciprocal
                # of the rowsum column
                osb = wk2.tile([128, 2, NQT, 65], F32, name="osb", tag="osb",
                               bufs=2)
                for h2 in range(2):
                    nc.vector.tensor_copy(osb[:, h2], st["ou"][h2])
                rr = wk2.tile([128, 2 * NQT, 1], F32, name="rr", tag="rr", bufs=2)
                nc.vector.reciprocal_approx_fast(
                    rr, osb.rearrange("p a b c -> p (a b) c")[:, :, 64:65])
                st["osb"], st["rr"] = osb, rr
                st["stk"] = wk2.tile([128, QB], BF16, name=f"stk{st['p']}",
                                     tag=f"stk{st['p']}", bufs=2)
                cur_stk[st["p"]] = st["stk"]

            def emit_norm(st, qt):
                onm = wk2.tile([128, 128], BF16, name="onm", tag="onm", bufs=4)
                for h2 in range(2):
                    nc.vector.tensor_scalar_mul(
                        onm[:, h2 * 64:(h2 + 1) * 64],
                        st["osb"][:, h2, qt, 0:64],
                        st["rr"][:, h2 * NQT + qt, :])
                st["onm"][qt] = onm

            def emit_tr(st, qt):
                onm = st["onm"].pop(qt)
                trp = py.tile([128, 128], BF16, name="trp", tag="y")
                nc.tensor.transpose(trp, onm, ident)
                nc.vector.tensor_copy(st["stk"][:, qt * 128:(qt + 1) * 128], trp)

            cur_ysb = [None]

            def emit_outproj_piece(qb, i, fast=False):
                # reads cur_stk at emission time: by the first piece (slot
                # LAG+2) both pairs' stk tiles for `qb` are registered.
                # fast mode (epilogue): spread yp over the now-idle sc PSUM
                # slots and alternate copy engines so the tail pipelines.
                qt, dc2 = divmod(i, 2)
                if dc2 == 0:
                    cur_ysb[0] = wk2.tile([128, D], F32, name="ysb", tag="ysb",
                                          bufs=2)
                ysb = cur_ysb[0]
                dsl = slice(dc2 * 512, (dc2 + 1) * 512)
                if fast and i % 2 == 0:
                    yp = psc.tile([128, 2, QB], F32, name="ype", tag="sc")
                    yp = yp[:, 0, :]
                else:
                    yp = py.tile([128, 512], F32, name="yp", tag="y")
                for p in range(NPAIR):
                    nc.tensor.matmul(yp, cur_stk[p][:, qt * 128:(qt + 1) * 128],
                                     wo_sb[:, p, dsl],
                                     start=(p == 0), stop=(p == NPAIR - 1))
                if fast and i % 2 == 1:
                    # ACT is idle once the last exp is done and can read PSUM
                    nc.scalar.copy(ysb[:, dsl], yp)
                else:
                    nc.vector.tensor_copy(ysb[:, dsl], yp)
                if dc2 == 1:
                    r0 = qb * QB + qt * 128
                    nc.sync.dma_start(out[r0:r0 + 128, :], ysb)

            # ---------- global task schedule -------------------------------
            KCOST, QCOST, VCOST = 4400, 4400, 2300
            # loop 0 = (qb0, p0): k chunks due before their sc; v tiles due
            # before their (lagged) av; pair-1 k0/q due late in loop 0
            add_task((0, 2), KCOST, lambda: project_k_chunk(0, 1))
            for s_ in range(2, 10):
                add_task((0, s_ + 5), VCOST, lambda s=s_: project_v_tile(s))
            add_task((0, 6), KCOST, lambda: project_k_chunk(0, 2))
            add_task((0, 10), KCOST, lambda: project_k_chunk(0, 3))
            add_task((0, 13), KCOST, lambda: project_k_chunk(1, 0))
            add_task((0, 14), QCOST, lambda: project_q(1, 0))
            for s_ in range(10, 16):
                due = (0, 15) if s_ == 10 else (1, s_ - 11)
                add_task(due, VCOST, lambda s=s_: project_v_tile(s))
            add_task((1, 2), KCOST, lambda: project_k_chunk(1, 1))
            add_task((1, 6), KCOST, lambda: project_k_chunk(1, 2))
            add_task((1, 10), KCOST, lambda: project_k_chunk(1, 3))
            add_task((1, 14), QCOST, lambda: project_q(0, 1))
            add_task((2, 14), QCOST, lambda: project_q(1, 1))
            add_task((3, 14), QCOST, lambda: project_q(0, 2))
            add_task((4, 14), QCOST, lambda: project_q(1, 2))
            add_task((5, 14), QCOST, lambda: project_q(0, 3))
            add_task((6, 14), QCOST, lambda: project_q(1, 3))
            tasks.sort(key=lambda t: t[0])
            taskq = deque(tasks)

            def make_state(p, qb):
                return {"p": p, "qb": qb, "exs": {}, "onm": {},
                        "ou": [pou.tile([128, NQT, 65], F32, name=f"ou{h2}",
                                        tag=f"ou{h2}", bufs=1)
                               for h2 in range(2)]}

            # ---------- prologue: k/v first (enc lands first), then q the
            # moment x block 0 arrives ------------------------------------
            project_k_chunk(0, 0, 0, 128)
            project_k_chunk(0, 0, 128, QB)
            project_v_tile(0)
            project_v_tile(1)
            project_q(0, 0)

            # ---------- main loops ----------------------------------------
            loops = [(qb, p) for qb in range(NQB) for p in range(NPAIR)]
            prev = None
            for L, (qb, p) in enumerate(loops):
                pump((L, -1), 0)
                outproj_qb = qb - 1 if (p == 0 and qb > 0) else None
                st = make_state(p, qb)
                emit_sc_exp(st, 0)
                last = L == len(loops) - 1
                debt = 0
                for kc in range(KC):
                    room = 1376 - debt
                    if kc < KC - 1:
                        emit_sc_exp(st, kc + 1)
                    if prev is not None and kc < LAG:
                        emit_av(prev, KC - LAG + kc)
                        room -= 520
                        if kc == LAG - 1:
                            emit_osb(prev)
                    if kc >= LAG:
                        emit_av(st, kc - LAG)
                        room -= 520
                    else:
                        room += 500
                    if prev is not None and LAG <= kc < LAG + 4:
                        emit_norm(prev, kc - LAG)
                        room -= 100
                    if prev is not None and LAG + 1 <= kc < LAG + 5:
                        emit_tr(prev, kc - LAG - 1)
                        room -= 200
                        if kc == LAG + 4:
                            prev = None
                    if outproj_qb is not None and LAG + 2 <= kc < LAG + 10:
                        emit_outproj_piece(outproj_qb, kc - LAG - 2)
                        room -= 1100
                    room = pump((L, kc), room)
                    if last and kc >= 11:
                        # catch-up: drain the av tail inside the loop so the
                        # epilogue starts almost immediately after exp(15)
                        emit_av(st, kc - 1)
                    debt = max(0, -room)
                prev = st

            # ---------- epilogue ------------------------------------------
            st = prev
            for kc in sorted(st["exs"]):
                emit_av(st, kc)
            emit_osb(st)
            for qt in range(NQT):
                emit_norm(st, qt)
                emit_tr(st, qt)
                emit_outproj_piece(NQB - 1, 2 * qt, fast=True)
                emit_outproj_piece(NQB - 1, 2 * qt + 1, fast=True)
            pump((99, 99), 0)

    nc.compile()
    return nc


def _bf16(a):
    return np.ascontiguousarray(a.astype(ml_dtypes.bfloat16))


def _host_prep(inputs):
    x = np.asarray(inputs["x"], np.float32)
    enc = np.asarray(inputs["encoder_output"], np.float32)
    Wq = np.asarray(inputs["Wq"], np.float32)
    bq = np.asarray(inputs["bq"], np.float32)
    Wk = np.asarray(inputs["Wk"], np.float32)
    bk = np.asarray(inputs["bk"], np.float32)
    Wv = np.asarray(inputs["Wv"], np.float32)
    Wo = np.asarray(inputs["Wo"], np.float32)

    xt_b = [_bf16(x[b].T) for b in range(B)]
    et_b = [_bf16(enc[b].T) for b in range(B)]

    in_maps = []
    for c in range(NCORES):
        b = c // 4
        hb = HPC * (c % 4)

        wq_c = Wq[hb:hb + 4].reshape(2, 2, DC, 128, HD)  # [pair, hw, dc, dp, e]
        wq_c = wq_c.transpose(3, 0, 2, 1, 4).reshape(128, NPAIR, DC, 128)
        wk_c = Wk[hb:hb + 4].reshape(2, 2, DC, 128, HD)
        wk_c = wk_c.transpose(3, 0, 2, 1, 4).reshape(128, NPAIR, DC, 128)
        wv_c = Wv[hb:hb + 4].reshape(4, DC, 128, HD)
        wv_c = wv_c.transpose(2, 1, 0, 3).reshape(128, DC, 256)
        wo_c = Wo[hb * HD:(hb + 4) * HD].reshape(2, 2, HD, D)  # [pair, hw, e, d]
        wo_c = wo_c.transpose(1, 2, 0, 3).reshape(128, NPAIR, D)
        bq_c = bq[hb:hb + 4].reshape(2, 2, HD).transpose(1, 2, 0).reshape(128, NPAIR)
        bk_c = bk[hb:hb + 4].reshape(2, 2, HD).transpose(1, 2, 0).reshape(128, NPAIR)

        in_maps.append({
            "xt": xt_b[b],
            "et": et_b[b],
            "wq": _bf16(wq_c),
            "wk": _bf16(wk_c),
            "wv": _bf16(wv_c),
            "wo": _bf16(wo_c),
            "bq": np.ascontiguousarray(bq_c),
            "bk": np.ascontiguousarray(bk_c),
        })
    return in_maps


def kernel(**inputs):
    if "nc" not in _CACHE:
        _CACHE["nc"] = _build_program()
    nc = _CACHE["nc"]

    in_maps = _host_prep(inputs)
    res = None
    for attempt in range(3):
        try:
            res = run_bass_kernel_spmd(nc, in_maps, core_ids=list(range(NCORES)))
            break
        except Exception:
            if attempt == 2:
                raise
            import time
            time.sleep(5)
    _CACHE["last_results"] = res

    bv = np.asarray(inputs["bv"], np.float32)
    Wo = np.asarray(inputs["Wo"], np.float32)
    bo = np.asarray(inputs["bo"], np.float32)
    const_d = bo + np.einsum("he,hed->d", bv,
                             Wo.reshape(H, HD, D)).astype(np.float32)

    out = np.empty((B, S, D), np.float32)
    for b in range(B):
        acc = res.results[4 * b]["out"].astype(np.float32).copy()
        for c in range(4 * b + 1, 4 * b + 4):
            acc += res.results[c]["out"]
        out[b] = acc + const_d
    return out
